# revision 9
# baseline (speedup 1.0000x reference)
"""Trainium2 Bass kernel for the AdaptiveGaussKronrod VJP quadrature problem.

Math (reference, flattened over N = S*15 = 1920 quadrature nodes):
    phi = sin(t (x) freqs)                  [N, D]
    Z   = phi @ W + b                       [N, D]
    G   = (h*wk)_n * cos(t (x) afreqs) * (1 - tanh(Z)^2)
    out = phi^T @ G                         [D, D]

Sharding: output-column parallel over 8 cores (J = D/8 = 512 columns each).
Core i needs W[:, cols], b[cols], afreqs[cols], full freqs. No collectives:
each core's [D, 512] output block is independent; host concatenates.

Per-core pipeline (Tile framework, bf16 matmuls / fp32 accumulation):
  pass 1 (GEMM1): phi_T tiles ([d, n] layout) generated by ScalarE Sin
    activation in n-blocks; Z accumulated in PSUM per n-row-tile; epilogue
    computes G tiles [n, 512] via Tanh / Sin(pi/2 - x) / DVE arithmetic.
  pass 2 (GEMM2): phi_N tiles ([n, d] layout) regenerated by ScalarE in
    d-column blocks; out accumulated in PSUM per d-row-tile; DMA to DRAM.
"""

import math

import numpy as np

D = 4096
S = 128
J = D // 8          # output columns per core
N = S * 15          # 1920 quadrature nodes
P = 128
KT = D // P         # 32 k-tiles over D
MT = N // P         # 15 m-tiles over N
OT = D // P         # 32 output row tiles

# phi_T n-blocks: 3 blocks x 640 cols (5 m-tiles each)
PT_BLK_M = 5
PT_BLK_W = PT_BLK_M * P          # 640
PT_NBLK = MT // PT_BLK_M         # 3
# phi_N d-col blocks: 8 blocks x 512 cols (4 o-tiles each)
PN_BLK_O = 4
PN_BLK_W = PN_BLK_O * P          # 512
PN_NBLK = OT // PN_BLK_O         # 8

_NODES_NEG = np.array([-0.9914553711208126, -0.9491079123427585, -0.8648644233597691,
                       -0.7415311855993945, -0.5860872354676911, -0.4058451513773972,
                       -0.20778495500789848, 0.0])
_WK_HALF = np.array([0.022935322010529224, 0.06309209262997856, 0.10479001032225019,
                     0.14065325971552592, 0.1690047266392679, 0.19035057806478542,
                     0.20443294007529889, 0.20948214108472782])
GK_NODES = np.concatenate([-_NODES_NEG[:-1][::-1], _NODES_NEG])  # [15]
GK_WK = np.concatenate([_WK_HALF[:-1][::-1], _WK_HALF])          # [15]


def _host_constants():
    edges = np.linspace(0.0, 1.0, S + 1, dtype=np.float32)
    a_s, b_s = edges[:-1], edges[1:]
    h = (b_s - a_s) / 2.0
    c = (a_s + b_s) / 2.0
    t = (c[:, None] + h[:, None] * GK_NODES[None, :].astype(np.float32)).reshape(-1)
    hw = (h[:, None] * GK_WK[None, :].astype(np.float32)).reshape(-1)
    return t.astype(np.float32), hw.astype(np.float32)


def _patch_act_tables():
    """Force Sin AND Tanh to resolve to one table set (silu_and_others) so
    the act-table-load pass emits a single load instead of thrashing
    between trig_and_small and exp_and_others on every Sin<->Tanh switch."""
    import concourse.bacc as bacc_mod
    from concourse import mybir

    if getattr(bacc_mod, "_act_tables_pinned", False):
        return
    orig = bacc_mod.get_activation_tables
    Sin = mybir.ActivationFunctionType.Sin
    Tanh = mybir.ActivationFunctionType.Tanh

    def patched(arch):
        tabs = orig(arch)
        out = {}
        for name, funcs in tabs.items():
            if (Sin in funcs) and (Tanh in funcs):
                out[name] = funcs
            else:
                out[name] = funcs - {Sin, Tanh}
        return out

    bacc_mod.get_activation_tables = patched
    bacc_mod._act_tables_pinned = True


def build_bass():
    """Build and compile the per-core Bass graph (identical on all 8 cores)."""
    from contextlib import ExitStack

    import concourse.bass as bass
    import concourse.tile as tile
    from concourse import bacc, mybir

    _patch_act_tables()

    f32 = mybir.dt.float32
    bf16 = mybir.dt.bfloat16
    Sin = mybir.ActivationFunctionType.Sin
    Tanh = mybir.ActivationFunctionType.Tanh

    nc = bacc.Bacc("TRN2", target_bir_lowering=False, debug=False,
                   enable_asserts=False)

    w_ext = nc.dram_tensor("w", [D, J], f32, kind="ExternalInput")
    freqs_ext = nc.dram_tensor("freqs", [D], f32, kind="ExternalInput")
    af_ext = nc.dram_tensor("af", [J], f32, kind="ExternalInput")
    b_ext = nc.dram_tensor("b", [J], f32, kind="ExternalInput")
    t_ext = nc.dram_tensor("t", [N], f32, kind="ExternalInput")
    hw_ext = nc.dram_tensor("hw", [N], f32, kind="ExternalInput")
    out_ext = nc.dram_tensor("out", [D, J], f32, kind="ExternalOutput")

    with tile.TileContext(nc) as tc, ExitStack() as ctx:
        consts = ctx.enter_context(tc.tile_pool(name="consts", bufs=1))
        stage = ctx.enter_context(tc.tile_pool(name="stage", bufs=4))
        wsp = ctx.enter_context(tc.tile_pool(name="ws", bufs=KT))
        phip = ctx.enter_context(tc.tile_pool(name="phi", bufs=64))
        work = ctx.enter_context(tc.tile_pool(name="work", bufs=2))
        gp = ctx.enter_context(tc.tile_pool(name="g", bufs=MT))
        zps = ctx.enter_context(
            tc.tile_pool(name="zpsum", bufs=6, space=bass.MemorySpace.PSUM))
        ops = ctx.enter_context(
            tc.tile_pool(name="opsum", bufs=2, space=bass.MemorySpace.PSUM))

        # ---- critical-path constants (first phiT tile needs these) ----
        t_bc = consts.tile([P, N], f32, tag="t_bc")
        nc.sync.dma_start(t_bc[:], t_ext.ap().partition_broadcast(P))
        f_pc = consts.tile([P, KT], f32, tag="f_pc")
        nc.sync.dma_start(f_pc[:], freqs_ext.ap().rearrange("(k p) -> p k", p=P))
        zero_c = consts.tile([P, 1], f32, tag="zero_c")
        nc.vector.memset(zero_c[:], 0.0)

        # ---- W shard: DMA f32, convert to bf16 (DVE) ----
        ws = []
        for k in range(KT):
            stg = stage.tile([P, J], f32, tag="stage512", name=f"wstg{k}")
            nc.sync.dma_start(stg[:], w_ext[k * P:(k + 1) * P, :])
            wb = wsp.tile([P, J], bf16, tag="ws", name=f"ws{k}")
            nc.vector.tensor_copy(wb[:], stg[:])
            ws.append(wb)

        # ---- ALL phiT blocks generated up-front in ScalarE program order.
        # ScalarE is in-order: epilogue tanh ops must not sit ahead of the
        # next block's phi generation, or the PE stalls at block edges.
        # The 64-slot phi pool paces generation via progressive release.
        phiT_blocks = []
        for blk in range(PT_NBLK):
            n0 = blk * PT_BLK_W
            phiT = []
            for k in range(KT):
                pt = phip.tile([P, PT_BLK_W], bf16, tag="phi",
                               name=f"pt{blk}_{k}")
                nc.scalar.activation(pt[:], t_bc[:, n0:n0 + PT_BLK_W], Sin,
                                     bias=zero_c[:], scale=f_pc[:, k:k + 1])
                phiT.append(pt)
            phiT_blocks.append(phiT)

        # ---- epilogue constants (needed from first epilogue, ~35us in) ----
        af_bc = consts.tile([P, J], f32, tag="af_bc")
        nc.sync.dma_start(af_bc[:], af_ext.ap().partition_broadcast(P))
        b_bc = consts.tile([P, J], f32, tag="b_bc")
        nc.sync.dma_start(b_bc[:], b_ext.ap().partition_broadcast(P))
        t_pc = consts.tile([P, MT], f32, tag="t_pc")
        nc.sync.dma_start(t_pc[:], t_ext.ap().rearrange("(m p) -> p m", p=P))
        hw_pc = consts.tile([P, MT], f32, tag="hw_pc")
        nc.sync.dma_start(hw_pc[:], hw_ext.ap().rearrange("(m p) -> p m", p=P))
        halfpi_c = consts.tile([P, 1], f32, tag="halfpi_c")
        nc.vector.memset(halfpi_c[:], math.pi / 2)
        tn_pc = consts.tile([P, MT], f32, tag="tn_pc")
        nc.vector.tensor_scalar_mul(tn_pc[:], t_pc[:], -1.0)
        # pass-2-only broadcast, deliberately after the W DMAs
        freqs_bc = consts.tile([P, D], f32, tag="freqs_bc")
        nc.sync.dma_start(freqs_bc[:], freqs_ext.ap().partition_broadcast(P))

        # ---- pass 1: GEMM1 (Z = phi @ Ws + b) and G epilogue ----
        def gen_phin_block(blk):
            c0 = blk * PN_BLK_W
            tiles = []
            for n in range(MT):
                pn = phip.tile([P, PT_BLK_W], bf16, tag="phi",
                               name=f"pn{blk}_{n}")
                nc.scalar.activation(pn[:, :PN_BLK_W],
                                     freqs_bc[:, c0:c0 + PN_BLK_W], Sin,
                                     bias=zero_c[:], scale=t_pc[:, n:n + 1])
                tiles.append(pn)
            return tiles

        phiN_blocks = {}
        g_tiles = [None] * MT
        for blk in range(PT_NBLK):
            phiT = phiT_blocks[blk]
            if blk == PT_NBLK - 1:
                # pre-generate two phiN blocks on ScalarE before the final
                # epilogue so pass 2 matmuls can start the moment G is done
                phiN_blocks[0] = gen_phin_block(0)
                phiN_blocks[1] = gen_phin_block(1)
            zt = [zps.tile([P, J], f32, tag="zpsum", name=f"zt{blk}_{i}")
                  for i in range(PT_BLK_M)]
            for k in range(KT):
                for ml in range(PT_BLK_M):
                    nc.tensor.matmul(zt[ml][:],
                                     lhsT=phiT[k][:, ml * P:(ml + 1) * P],
                                     rhs=ws[k][:],
                                     start=(k == 0), stop=(k == KT - 1))
            for ml in range(PT_BLK_M):
                m = blk * PT_BLK_M + ml
                z = work.tile([P, J], f32, tag="z")
                nc.vector.tensor_add(z[:], zt[ml][:], b_bc[:])
                nc.scalar.activation(z[:], z[:], Tanh, bias=zero_c[:])
                c = work.tile([P, J], f32, tag="c")
                nc.scalar.activation(c[:], af_bc[:], Sin,
                                     scale=tn_pc[:, m:m + 1], bias=halfpi_c[:])
                s = work.tile([P, J], f32, tag="s")
                nc.vector.tensor_mul(s[:], z[:], z[:])
                u = work.tile([P, J], f32, tag="u")
                nc.vector.tensor_scalar(u[:], s[:], -1.0, 1.0,
                                        mybir.AluOpType.mult, mybir.AluOpType.add)
                v = work.tile([P, J], f32, tag="v")
                nc.vector.tensor_mul(v[:], c[:], u[:])
                g = gp.tile([P, J], bf16, tag="g")
                nc.vector.tensor_scalar_mul(g[:], v[:], hw_pc[:, m:m + 1])
                g_tiles[m] = g

        # ---- pass 2: GEMM2 (out = phi^T @ G) ----
        for blk in range(PN_NBLK):
            phiN = phiN_blocks.get(blk) or gen_phin_block(blk)
            for ol in range(PN_BLK_O):
                o = blk * PN_BLK_O + ol
                op = ops.tile([P, J], f32, tag="opsum", name=f"op{o}")
                for n in range(MT):
                    nc.tensor.matmul(op[:],
                                     lhsT=phiN[n][:, ol * P:(ol + 1) * P],
                                     rhs=g_tiles[n][:],
                                     start=(n == 0), stop=(n == MT - 1))
                ostg = stage.tile([P, J], f32, tag="stage512", name=f"ostg{o}")
                nc.vector.tensor_copy(ostg[:], op[:])
                nc.sync.dma_start(out_ext[o * P:(o + 1) * P, :], ostg[:])

    nc.compile()
    return nc


_CACHE = {}


def _get_nc():
    if "nc" not in _CACHE:
        _CACHE["nc"] = build_bass()
    return _CACHE["nc"]


def kernel(W, b, freqs, afreqs):
    from concourse.bass_utils import run_bass_kernel_spmd

    W = np.ascontiguousarray(np.asarray(W, dtype=np.float32))
    b = np.asarray(b, dtype=np.float32)
    freqs = np.ascontiguousarray(np.asarray(freqs, dtype=np.float32))
    afreqs = np.asarray(afreqs, dtype=np.float32)
    t, hw = _host_constants()

    nc = _get_nc()
    in_maps = []
    for i in range(8):
        sl = slice(i * J, (i + 1) * J)
        in_maps.append({
            "w": np.ascontiguousarray(W[:, sl]),
            "freqs": freqs,
            "af": np.ascontiguousarray(afreqs[sl]),
            "b": np.ascontiguousarray(b[sl]),
            "t": t,
            "hw": hw,
        })
    res = run_bass_kernel_spmd(nc, in_maps, core_ids=list(range(8)))
    return np.concatenate([res.results[i]["out"] for i in range(8)], axis=1)
